# revision 14
# baseline (speedup 1.0000x reference)
"""Gaussian L1-distance attention kernel for Trainium2 (8 NeuronCores).

Computes y[b,s,i,j] = exp(-(sum_d |x[b,i,d]-x[b,j,d]|)^2 / (2*sigma_s^2))
for x [4,2048,3] f32, sigmas [8] f32 -> y [4,8,2048,2048] f32 (512MB).

Sharding: 8 cores; core c handles batch b=c//2, row half rh=c%2
(rows rh*1024..rh*1024+1023). Per-core output [8,1024,2048] = 64MB.

All per-core inputs are packed host-side into one [128, 6176] tensor
(x[b].T replicated across partitions | per-tile row coords | sigmas)
so the whole input arrives in a single DMA -> every VectorE
tensor_scalar has at most one sync wait (the TS ISA struct has a
single wait slot).

Per 128-row tile: dist via 3 fused |xb_d - xi_d| tensor_scalar ops
(subtract + abs_max vs 0) + 2 adds + 1 square on VectorE; 8 exps on
ScalarE with per-partition scale AP (-1/(2 sigma^2), computed on-chip
from the sigmas input); 8x 1MB contiguous DMA stores.
"""

import numpy as np

B, N, D, S = 4, 2048, 3, 8
NCORES = 8
ROWS = N // 2          # 1024 rows per core
NTILES = ROWS // 128   # 8 tiles of 128 rows
XI_OFF = D * N         # 6144
SIG_OFF = XI_OFF + NTILES * D  # 6168
XC_W = SIG_OFF + S     # 6176

_cached = None
TRACE_KW: dict = {}
LAST_RESULT = None


def _build():
    from concourse import mybir
    from concourse.bacc import Bacc
    from concourse.tile import TileContext

    f32 = mybir.dt.float32
    Alu = mybir.AluOpType
    Act = mybir.ActivationFunctionType

    # Bacc (not Bass): its compile() runs generate_event_semaphores, which
    # splits instructions with >1 sync wait (TRN2 allows one per inst).
    nc = Bacc()
    xc = nc.dram_tensor("xc", [128, XC_W], f32, kind="ExternalInput")
    y = nc.dram_tensor("y", [S, ROWS, N], f32, kind="ExternalOutput")

    with TileContext(nc) as tc:
        with (
            tc.tile_pool(name="const", bufs=1) as cpool,
            tc.tile_pool(name="absd", bufs=3) as apool,
            tc.tile_pool(name="mid", bufs=2) as mpool,
            tc.tile_pool(name="distp", bufs=2) as dpool,
            tc.tile_pool(name="sqp", bufs=2) as qpool,
            tc.tile_pool(name="outp", bufs=8) as opool,
        ):
            xcs = cpool.tile([128, XC_W], f32)
            nc.sync.dma_start(out=xcs[:], in_=xc[:])
            sig = xcs[:, SIG_OFF:SIG_OFF + S]
            # neg_inv[:, s] = -1/(2*sigma_s^2)
            s2 = cpool.tile([128, S], f32)
            nc.vector.tensor_tensor(out=s2[:], in0=sig, in1=sig, op=Alu.mult)
            s2n = cpool.tile([128, S], f32)
            nc.vector.tensor_scalar_mul(s2n[:], s2[:], -2.0)
            neg_inv = cpool.tile([128, S], f32)
            nc.vector.reciprocal(out=neg_inv[:], in_=s2n[:])

            for t in range(NTILES):
                # |xb_d - xi_d| = max(xb_d - xi_d, xi_d - xb_d), two DVE ops
                # per d (abs/abs_max are not valid tensor_scalar ALU ops).
                a0 = apool.tile([128, N], f32, tag="a")
                a1 = apool.tile([128, N], f32, tag="a")
                a2 = apool.tile([128, N], f32, tag="a")
                for d, a in enumerate((a0, a1, a2)):
                    xb_d = xcs[:, d * N:(d + 1) * N]
                    xi_d = xcs[:, XI_OFF + t * D + d:XI_OFF + t * D + d + 1]
                    neg = mpool.tile([128, N], f32, tag="neg")
                    nc.vector.tensor_scalar(
                        neg[:], xb_d, xi_d, -1.0, Alu.subtract, Alu.mult,
                    )
                    nc.vector.scalar_tensor_tensor(
                        out=a[:], in0=xb_d, scalar=xi_d, in1=neg[:],
                        op0=Alu.subtract, op1=Alu.max,
                    )
                s01 = mpool.tile([128, N], f32, tag="s01")
                nc.vector.tensor_tensor(out=s01[:], in0=a0[:], in1=a1[:], op=Alu.add)
                dist = dpool.tile([128, N], f32, tag="dist")
                nc.vector.tensor_tensor(out=dist[:], in0=s01[:], in1=a2[:], op=Alu.add)
                sq = qpool.tile([128, N], f32, tag="sq")
                nc.scalar.square(out=sq[:], in_=dist[:])

                for s in range(S):
                    o = opool.tile([128, N], f32, tag="o")
                    nc.scalar.activation(
                        out=o[:], in_=sq[:], func=Act.Exp,
                        scale=neg_inv[:, s:s + 1],
                    )
                    nc.sync.dma_start(
                        out=y[s, t * 128:(t + 1) * 128, :], in_=o[:]
                    )
    # Run Bacc's compile pipeline (alloc_regs, generate_event_semaphores —
    # the pass that splits >1-wait instructions for TRN2's 1-wait ISA limit).
    nc.finalize()
    return nc


def _pack_core_input(xb: np.ndarray, rows: np.ndarray, sigmas: np.ndarray) -> np.ndarray:
    """xb: [N, D] full batch slice; rows: [ROWS, D] this core's rows."""
    out = np.empty((128, XC_W), dtype=np.float32)
    out[:, :XI_OFF] = xb.T.reshape(1, D * N)
    # xi_all[p, t*D + d] = rows[t*128 + p, d]
    out[:, XI_OFF:SIG_OFF] = rows.reshape(NTILES, 128, D).transpose(1, 0, 2).reshape(128, NTILES * D)
    out[:, SIG_OFF:] = sigmas[None, :]
    return out


def kernel(x: np.ndarray, sigmas: np.ndarray) -> np.ndarray:
    global _cached
    from concourse import bass_utils

    x = np.ascontiguousarray(np.asarray(x, dtype=np.float32))
    sigmas = np.ascontiguousarray(np.asarray(sigmas, dtype=np.float32))

    if _cached is None:
        _cached = _build()
    nc = _cached

    in_maps = []
    for c in range(NCORES):
        b, rh = c // 2, c % 2
        in_maps.append({
            "xc": _pack_core_input(
                x[b], x[b, rh * ROWS:(rh + 1) * ROWS, :], sigmas
            ),
        })

    res = bass_utils.run_bass_kernel_spmd(
        nc, in_maps, core_ids=list(range(NCORES)), **TRACE_KW
    )
    global LAST_RESULT
    LAST_RESULT = res

    out = np.empty((B, S, N, N), dtype=np.float32)
    for c in range(NCORES):
        b, rh = c // 2, c % 2
        out[b, :, rh * ROWS:(rh + 1) * ROWS, :] = res.results[c]["y"]
    return out


# revision 15
# speedup vs baseline: 1.3915x; 1.3915x over previous
"""Gaussian L1-distance attention kernel for Trainium2 (8 NeuronCores).

Computes y[b,s,i,j] = exp(-(sum_d |x[b,i,d]-x[b,j,d]|)^2 / (2*sigma_s^2))
for x [4,2048,3] f32, sigmas [8] f32 -> y [4,8,2048,2048] f32 (512MB).

The distance matrix is symmetric, so each core computes only the upper
triangle (53% of the elements) and the host mirrors the lower triangle
during unsharding (bit-exact: |a-b| and the downstream ops are symmetric).

Sharding (SPMD-uniform): core c -> batch b=c//2, sigma half h=c%2.
Every core processes the same 16 row-tiles (tile k: rows r*128..r*128+127
with r=15-k, columns r*128..2047, width W_k = 128*(k+1)) for its 4 sigmas.
Identical shapes/offsets on every core; only input data differs.

Per tile: |xb_d - xi_d| for d=0,1 on VectorE (tensor_scalar (xb-xi)*-1
fused, then scalar_tensor_tensor max(xb-xi, neg) -- abs is not a valid
DVE ALU op); d=2 on ScalarE (Abs activation, bias = -xi packed by host).
Sum + square -> 4 exps on ScalarE with per-partition scale AP
(-1/(2 sigma^2) computed on-chip) -> packed DMA stores.

Built with Bacc (not Bass): its finalize() runs generate_event_semaphores,
which splits instructions carrying more than one sync wait (TRN2 ISA
allows a single wait per compute instruction).
"""

import numpy as np

B, N, D, S = 4, 2048, 3, 8
NCORES = 8
NTILES = 16
S_LOC = S // 2                       # 4 sigmas per core
WIDTHS = [128 * (k + 1) for k in range(NTILES)]   # ascending, r = 15-k
PACKW = sum(WIDTHS)                  # 17408
WOFF = [sum(WIDTHS[:k]) for k in range(NTILES)]
XI_OFF = D * N                       # 6144: xi_all [16*3]
XN_OFF = XI_OFF + NTILES * D         # 6192: -xi (d=2) [16]
SIG_OFF = XN_OFF + NTILES            # 6208: this core's 4 sigmas
XC_W = SIG_OFF + S_LOC               # 6212

_cached = None
TRACE_KW: dict = {}
LAST_RESULT = None


def _build():
    from concourse import mybir
    from concourse.bacc import Bacc
    from concourse.tile import TileContext

    f32 = mybir.dt.float32
    Alu = mybir.AluOpType
    Act = mybir.ActivationFunctionType

    nc = Bacc()
    xc = nc.dram_tensor("xc", [128, XC_W], f32, kind="ExternalInput")
    y = nc.dram_tensor("y", [S_LOC, 128, PACKW], f32, kind="ExternalOutput")

    with TileContext(nc) as tc:
        with (
            tc.tile_pool(name="const", bufs=1) as cpool,
            tc.tile_pool(name="absd", bufs=3) as apool,
            tc.tile_pool(name="negp", bufs=2) as npool,
            tc.tile_pool(name="a2p", bufs=2) as a2pool,
            tc.tile_pool(name="mid", bufs=2) as mpool,
            tc.tile_pool(name="distp", bufs=2) as dpool,
            tc.tile_pool(name="sqp", bufs=2) as qpool,
            tc.tile_pool(name="outp", bufs=6) as opool,
        ):
            xcs = cpool.tile([128, XC_W], f32)
            nc.sync.dma_start(out=xcs[:], in_=xc[:])
            sig = xcs[:, SIG_OFF:SIG_OFF + S_LOC]
            # neg_inv[:, sl] = -1/(2*sigma^2)
            s2 = cpool.tile([128, S_LOC], f32)
            nc.vector.tensor_tensor(out=s2[:], in0=sig, in1=sig, op=Alu.mult)
            s2n = cpool.tile([128, S_LOC], f32)
            nc.vector.tensor_scalar_mul(s2n[:], s2[:], -2.0)
            neg_inv = cpool.tile([128, S_LOC], f32)
            nc.vector.reciprocal(out=neg_inv[:], in_=s2n[:])

            for k in range(NTILES):
                w = WIDTHS[k]
                c0 = (NTILES - 1 - k) * 128
                xi = [xcs[:, XI_OFF + k * D + d:XI_OFF + k * D + d + 1]
                      for d in range(D)]
                xb = [xcs[:, d * N + c0:d * N + c0 + w] for d in range(D)]

                # d=0,1 on VectorE
                aa = []
                for d in range(2):
                    neg = npool.tile([128, w], f32, tag="neg")
                    nc.vector.tensor_scalar(
                        neg[:], xb[d], xi[d], -1.0, Alu.subtract, Alu.mult,
                    )
                    a = apool.tile([128, w], f32, tag="a")
                    nc.vector.scalar_tensor_tensor(
                        out=a[:], in0=xb[d], scalar=xi[d], in1=neg[:],
                        op0=Alu.subtract, op1=Alu.max,
                    )
                    aa.append(a)
                # d=2 on ScalarE: |xb2 - xi2| = Abs(xb2 + (-xi2))
                a2 = a2pool.tile([128, w], f32, tag="a2")
                nc.scalar.activation(
                    out=a2[:], in_=xb[2], func=Act.Abs,
                    bias=xcs[:, XN_OFF + k:XN_OFF + k + 1],
                )
                s01 = mpool.tile([128, w], f32, tag="s01")
                nc.vector.tensor_tensor(out=s01[:], in0=aa[0][:], in1=aa[1][:], op=Alu.add)
                dist = dpool.tile([128, w], f32, tag="dist")
                nc.vector.tensor_tensor(out=dist[:], in0=s01[:], in1=a2[:], op=Alu.add)
                sq = qpool.tile([128, w], f32, tag="sq")
                nc.scalar.square(out=sq[:], in_=dist[:])

                for sl in range(S_LOC):
                    o = opool.tile([128, w], f32, tag="o")
                    nc.scalar.activation(
                        out=o[:], in_=sq[:], func=Act.Exp,
                        scale=neg_inv[:, sl:sl + 1],
                    )
                    nc.sync.dma_start(
                        out=y[sl, :, WOFF[k]:WOFF[k] + w], in_=o[:]
                    )
    nc.finalize()
    return nc


def _pack_core_input(xb: np.ndarray, sig4: np.ndarray) -> np.ndarray:
    """xb: [N, D] batch slice; sig4: this core's 4 sigmas."""
    out = np.empty((128, XC_W), dtype=np.float32)
    out[:, :XI_OFF] = xb.T.reshape(1, D * N)
    # tile k uses rows r*128..r*128+127, r = 15-k
    rows = xb.reshape(NTILES, 128, D)[::-1]           # [k, p, d]
    out[:, XI_OFF:XN_OFF] = rows.transpose(1, 0, 2).reshape(128, NTILES * D)
    out[:, XN_OFF:SIG_OFF] = -rows[:, :, 2].T         # [p, k]
    out[:, SIG_OFF:] = sig4[None, :]
    return out


def kernel(x: np.ndarray, sigmas: np.ndarray) -> np.ndarray:
    global _cached, LAST_RESULT
    from concourse import bass_utils

    x = np.ascontiguousarray(np.asarray(x, dtype=np.float32))
    sigmas = np.ascontiguousarray(np.asarray(sigmas, dtype=np.float32))

    if _cached is None:
        _cached = _build()
    nc = _cached

    in_maps = []
    for c in range(NCORES):
        b, h = c // 2, c % 2
        in_maps.append({
            "xc": _pack_core_input(x[b], sigmas[h * S_LOC:(h + 1) * S_LOC]),
        })

    res = bass_utils.run_bass_kernel_spmd(
        nc, in_maps, core_ids=list(range(NCORES)), **TRACE_KW
    )
    LAST_RESULT = res

    out = np.empty((B, S, N, N), dtype=np.float32)
    for c in range(NCORES):
        b, h = c // 2, c % 2
        yl = res.results[c]["y"]                      # [S_LOC, 128, PACKW]
        for k in range(NTILES):
            r = NTILES - 1 - k
            w = WIDTHS[k]
            out[b, h * S_LOC:(h + 1) * S_LOC, r * 128:(r + 1) * 128, r * 128:] = (
                yl[:, :, WOFF[k]:WOFF[k] + w]
            )
    # mirror the lower triangle (bit-exact by symmetry)
    for r in range(NTILES - 1):
        src = out[:, :, r * 128:(r + 1) * 128, (r + 1) * 128:]
        out[:, :, (r + 1) * 128:, r * 128:(r + 1) * 128] = src.swapaxes(-1, -2)
    return out
